# revision 76
# baseline (speedup 1.0000x reference)
"""Trainium2 Bass kernel for nn_LiveNet_20504173871714 (dense MLP).

    out = relu(relu(x @ W1.T + b1) @ W2.T + b2)
    x: [4096, 2048] f32, W1: [8192, 2048], W2: [2048, 8192]

Strategy: data-parallel over batch across 8 NeuronCores (512 rows each);
no collectives. Everything runs in fp8e4m3 with DoubleRow matmuls
(0.5 cyc/row) and fp32 PSUM accumulation:

    GEMM1: hiddenT[m, b] = sum_i W1q[m, i] xq[b, i]   (W1 pair-tiles stationary)
    GEMM2: out[b, o]     = sum_j hq[j, b] W2q[o, j]   (hidden stationary, W2 moving,
                                                       so each weight load feeds 4 MMs)

fp8 quantization of x is the one non-averaging error term (W1 >= 0
propagates the row-sum of x's rounding error coherently through both
layers). It is cancelled with a rank-1 correction w1rm[m] * s[b],
where s[b] = sum_i (x - fp8(x))[b, i] and w1rm = W1.mean(axis=1)
(host-computed during input prep). The DVE pre-writes it into each
PSUM group before the matmuls (first matmul start=False accumulates
onto it — verified on HW), so it costs zero PE time; the "fp8dr" mode
instead folds it into a 9th aug k-tile on the PE (64 extra matmuls,
~19us slower, kept as fallback). Uncorrected all-fp8 would be 2.3e-2
l2 err — over the gate; corrected is ~2e-3 (dominated by the bf16
output cast).

W2 (fp8, 16 MiB) is cached in SBUF across both batch halves of GEMM2;
half its chunks prefetch under GEMM1's compute, the rest stream
just-in-time during GEMM2's first pass. All DRAM planes are stored
pre-arranged in SBUF layout so every load is one DMA of fully
contiguous per-partition runs. ReLU+bias eviction runs on ScalarE; the
output is written bf16 (cast back to f32 on host).

Measured (hardware For_i-loop reps-differencing, thermally-controlled
interleaved A/B): ~240-262 us/iteration vs the 300 us float32r
baseline; the fp8-DR matmul floor for this decomposition is ~218
us/core (the moving operand streams 2 B/partition/cycle — measured;
the CoreSim 0.5 cyc/row model is 2x optimistic).
"""

import numpy as np
import ml_dtypes

N_IN, N_MID, N_OUT, BATCH = 2048, 8192, 2048, 4096
N_CORES = 8
B = BATCH // N_CORES  # 512 rows per core
P = 128

G1_DT = "fp8drv"
G2_DT = "fp8drf"

IT = N_IN // (2 * P) + 1   # 8 real DR k-tiles + 1 aug (rank-1 x-correction)
JT2 = N_MID // (2 * P)     # 32 DR k-tiles in GEMM2
MG = 4                     # m-tiles per PSUM group in GEMM1
AUG_SCALE = 8.0

_CACHE = {}
_FP8 = ml_dtypes.float8_e4m3


def _q8(a):
    return np.asarray(a, dtype=np.float32).astype(_FP8)


def _build(g1_dt=G1_DT, g2_dt=G2_DT, enable_asserts=False, reps=1,
           hw_loop=0, split_start=False, g2_banks=8, tune=0):
    """Build + compile the per-core Bass module (cached).

    hw_loop>0 builds a timing variant: the whole computation body wrapped
    in a tc.For_i hardware loop of that many iterations, with all big
    tensors as Internal DRAM (no host upload) — wall-clock then ~= device
    time, for reps-differencing benchmarks."""
    key = (g1_dt, g2_dt, enable_asserts, reps, hw_loop, split_start,
           g2_banks, tune)
    if key in _CACHE:
        return _CACHE[key]

    import concourse.bass as bass
    import concourse.mybir as mybir
    import concourse.tile as tile
    from concourse import bacc
    from concourse.bass import ds, ts
    from contextlib import ExitStack, nullcontext

    # "fp8dr": rank-1 x-correction via a 9th aug k-tile on the PE.
    # "fp8drv": correction pre-written into PSUM by the DVE before each
    # matmul group (first matmul start=False accumulates onto it).
    assert g1_dt in ("fp8dr", "fp8drv")
    g1_dve = g1_dt == "fp8drv"
    ITk = IT - 1 if g1_dve else IT
    g2_flip = g2_dt == "fp8drf"
    fp8 = mybir.dt.float8e4
    f32 = mybir.dt.float32
    relu = mybir.ActivationFunctionType.Relu
    DR = mybir.MatmulPerfMode.DoubleRow

    nc = bacc.Bacc("TRN2", target_bir_lowering=False, debug=False,
                   enable_asserts=enable_asserts)

    kin = "Internal" if hw_loop else "ExternalInput"
    kout = "Internal" if hw_loop else "ExternalOutput"

    # All weight/activation planes are stored in DRAM pre-arranged in their
    # SBUF layout, so every load is one DMA with fully contiguous
    # per-partition runs.
    bf16 = mybir.dt.bfloat16
    # xT[p, it, q, b] = xq[b, it*256 + q*128 + p]   (+ aug plane it=8)
    xT = nc.dram_tensor("xT", [P, ITk, 2, B], fp8, kind=kin).ap()
    # w1T[g, p, it, q, ml] = W1q[g*512 + ml, it*256 + q*128 + p]
    w1T = nc.dram_tensor("w1T", [16, P, ITk, 2, MG * P], fp8, kind=kin).ap()
    if g1_dve:
        sbc = nc.dram_tensor("sbc", [P, B], bf16, kind=kin).ap()
        w1rms = nc.dram_tensor("w1rms", [P, N_MID // P], f32, kind=kin).ap()
    # w2T[c, p, tt, q, o] = W2q[o, (c*4 + tt)*256 + q*128 + p]
    w2T = nc.dram_tensor("w2T", [8, P, 4, 2, N_OUT], fp8, kind=kin).ap()
    b1s = nc.dram_tensor("b1s", [P, N_MID // P], f32, kind=kin).ap()
    b2s = nc.dram_tensor("b2s", [P, N_OUT // P], f32, kind=kin).ap()
    if g2_flip:
        outT = nc.dram_tensor("outF", [B, N_OUT], bf16, kind=kout).ap()
    else:
        outT = nc.dram_tensor("outT", [N_OUT, B], f32, kind=kout).ap()
    if hw_loop:
        sig = nc.dram_tensor("sig", [1, 4], f32, kind="ExternalOutput").ap()

    with tile.TileContext(nc) as tc, ExitStack() as ctx:
        const = ctx.enter_context(tc.tile_pool(name="const", bufs=1))
        xpool = ctx.enter_context(tc.tile_pool(name="xpool", bufs=1))
        hpool = ctx.enter_context(tc.tile_pool(name="hpool", bufs=JT2))
        w1pool = ctx.enter_context(tc.tile_pool(name="w1pool", bufs=3))
        w2pool = ctx.enter_context(
            tc.tile_pool(name="w2pool", bufs=(8 if g2_flip else 12)))
        opool = ctx.enter_context(tc.tile_pool(name="opool", bufs=4))
        psum = ctx.enter_context(tc.tile_pool(name="psum", bufs=8, space="PSUM"))

        b1_sb = const.tile([P, N_MID // P], f32, name="b1_sb")
        b2_sb = const.tile([P, N_OUT // P], f32, name="b2_sb")
        if g1_dve:
            # sbc/w1rm gate the first DVE psum pre-write -> load them first
            sbc_sb = const.tile([P, B], bf16, name="sbc_sb")
            nc.sync.dma_start(sbc_sb[:], sbc[:, :])
            w1rm_sb = const.tile([P, N_MID // P], f32, name="w1rm_sb")
            nc.sync.dma_start(w1rm_sb[:], w1rms[:, :])
        if not tune:
            nc.sync.dma_start(b1_sb[:], b1s[:, :])
            nc.sync.dma_start(b2_sb[:], b2s[:, :])
        if tune:
            # HAM pre-warm fodder: const zero tile for dummy matmuls
            wtile = const.tile([P, B], fp8, name="warm_t")
            nc.vector.memset(wtile[:], 0)

        loop_cm = tc.For_i(0, hw_loop) if hw_loop else nullcontext()
        with loop_cm:
          for rep in range(reps):
            # x^T DR pair tiles resident in SBUF: [128, 9, 2, 512], one DMA
            # split so the first matmul can start after a ~128 KiB DMA
            if tune:
                # ~10 dummy matmuls fill the PE during the startup DMA
                # window so the HAM clock-gate is warm for the real work
                wps = psum.tile([P, B], f32, tag="ps", name=f"warm_ps_{rep}")
                for i in range(10):
                    nc.tensor.matmul(wps[:], wtile[:, ts(0, P)], wtile[:],
                                     start=(i == 0), stop=(i == 9))
            xtile = xpool.tile([P, ITk, 2, B], fp8, tag="xT", name="xT_all")
            if split_start:
                nc.sync.dma_start(xtile[:, 0:1, :, :], xT[:, 0:1, :, :])
                nc.sync.dma_start(xtile[:, 1:, :, :], xT[:, 1:, :, :])
            else:
                nc.sync.dma_start(xtile[:], xT[:, :, :, :])
            if tune and rep == 0:
                nc.sync.dma_start(b1_sb[:], b1s[:, :])
                if not g2_flip:
                    nc.sync.dma_start(b2_sb[:], b2s[:, :])
            xts = [xtile[:, it, :, :] for it in range(ITk)]

            # W2 cached in SBUF for the whole of GEMM2 (both batch halves),
            # in 4-k-tile chunks (one DMA each). GEMM1 is the DMA-heavy
            # phase (W1 streams 18 MiB), so only 2 chunks prefetch near its
            # end; the rest stream just-in-time during GEMM2's first pass.
            W2C = 4                  # k-tiles per chunk
            NCH = JT2 // W2C         # 8 chunks
            PRE = 6 if tune else 4   # chunks prefetched during GEMM1
            if g2_flip:
                w2ch = [w2pool.tile([P, W2C, 2, N_OUT], fp8, tag="w2",
                                    name=f"w2c_{c}") for c in range(NCH)]

                def w2_fetch(c):
                    nc.sync.dma_start(w2ch[c][:], w2T[c, :, :, :, :])

            # GEMM1 + ReLU -> hiddenT pair tiles [128, 2, 512] fp8
            hts = [hpool.tile([P, 2, B], fp8, tag="hid", name=f"hid_{t}")
                   for t in range(JT2)]
            for mtg in range(N_MID // (MG * P)):
                psums = [psum.tile([P, B], f32, tag="ps", name=f"ps1_{mtg}_{s}")
                         for s in range(MG)]
                # one DMA per m-group: all k-tiles of this 512-col W1 slab
                blk = w1pool.tile([P, ITk, 2, MG * P], fp8, tag="w1",
                                  name=f"w1_{mtg}")
                if split_start and mtg == 0:
                    nc.sync.dma_start(blk[:, 0:1, :, :], w1T[0, :, 0:1, :, :])
                    nc.sync.dma_start(blk[:, 1:, :, :], w1T[0, :, 1:, :, :])
                else:
                    nc.sync.dma_start(blk[:], w1T[mtg, :, :, :, :])
                if g1_dve:
                    for s in range(MG):
                        mt = mtg * MG + s
                        nc.vector.tensor_scalar_mul(
                            psums[s][:], sbc_sb[:], w1rm_sb[:, mt:mt + 1])
                for it in range(ITk):
                    for s in range(MG):
                        nc.tensor.matmul(psums[s][:], blk[:, it, :, ts(s, P)],
                                         xts[it][:],
                                         start=(it == 0 and not g1_dve),
                                         stop=(it == ITk - 1),
                                         perf_mode=DR)
                if g2_flip and mtg >= 16 - 2 * PRE and mtg % 2 == 1:
                    w2_fetch((mtg - (16 - 2 * PRE)) // 2)
                for s in range(MG):
                    mt = mtg * MG + s
                    nc.scalar.activation(hts[mt // 2][:, mt % 2, :],
                                         psums[s][:], relu,
                                         bias=b1_sb[:, mt:mt + 1])

            # GEMM2 + ReLU -> out
            if g2_flip:
                # hidden stationary / W2 moving; psum tiles are [b, o].
                # psum groups per batch block(s): g2_banks=4 -> one 128-row
                # block per group (evictions overlap the next group's MMs);
                # g2_banks=8 -> two blocks per group.
                nbt = g2_banks // 4
                for bg in range(4 // nbt):
                    psums = [psum.tile([P, MG * P], f32, tag="ps",
                                       name=f"psf_{bg}_{k}")
                             for k in range(g2_banks)]
                    for t in range(JT2):
                        if bg == 0 and t % W2C == 0 and t // W2C + PRE < NCH:
                            w2_fetch(t // W2C + PRE)
                        for bi in range(nbt):
                            bt = bg * nbt + bi
                            lhs = hts[t][:, :, ts(bt, P)]
                            for ob in range(4):
                                nc.tensor.matmul(
                                    psums[bi * 4 + ob][:], lhs,
                                    w2ch[t // W2C][:, t % W2C, :,
                                                   ds(ob * MG * P, MG * P)],
                                    start=(t == 0), stop=(t == JT2 - 1),
                                    perf_mode=DR)
                    for bi in range(nbt):
                        for ob in range(4):
                            bt = bg * nbt + bi
                            o_sb = opool.tile([P, MG * P], bf16, tag="out",
                                              name=f"out_{bt}_{ob}")
                            nc.scalar.activation(o_sb[:],
                                                 psums[bi * 4 + ob][:], relu)
                            nc.sync.dma_start(
                                outT[ds(bt * P, P), ds(ob * MG * P, MG * P)],
                                o_sb[:])
                continue

            # Fallback (b2 != 0): W2 stationary, hidden moving; outT[o, b].
            for otg in range(N_OUT // (MG * P)):
                psums = [psum.tile([P, B], f32, tag="ps", name=f"ps2_{otg}_{s}")
                         for s in range(MG)]
                for jt in range(JT2):
                    blk = w2pool.tile([P, 2, MG * P], fp8, tag="w2",
                                      name=f"w2_{otg}_{jt}")
                    nc.sync.dma_start(
                        blk[:],
                        w2T[jt // 4, :, jt % 4, :, ds(otg * MG * P, MG * P)])
                    for s in range(MG):
                        nc.tensor.matmul(psums[s][:], blk[:, :, ts(s, P)],
                                         hts[jt][:],
                                         start=(jt == 0), stop=(jt == JT2 - 1),
                                         perf_mode=DR)
                for s in range(MG):
                    ot = otg * MG + s
                    o_sb = opool.tile([P, B], f32, tag="out", name=f"out_{ot}")
                    nc.scalar.activation(o_sb[:], psums[s][:], relu,
                                         bias=b2_sb[:, ot:ot + 1])
                    nc.sync.dma_start(outT[ts(ot, P), :], o_sb[:])

        if hw_loop:
            nc.sync.dma_start(sig[:, :], b1_sb[0:1, 0:4])

    nc.compile()
    _CACHE[key] = nc
    return nc


def _dr_pairs(a2d):
    """[K, N] -> [K//256, 128, 2, N] with k = it*256 + q*128 + p."""
    k, n = a2d.shape
    return np.ascontiguousarray(
        a2d.reshape(k // 256, 2, P, n).transpose(0, 2, 1, 3))


def _prep_inputs(x, W1, b1, W2, b2, g1_dt=G1_DT, g2_dt=G2_DT):
    x = np.asarray(x, dtype=np.float32)
    W1 = np.asarray(W1, dtype=np.float32)
    W2 = np.asarray(W2, dtype=np.float32)

    g1_dve = g1_dt == "fp8drv"
    ITk = IT - 1 if g1_dve else IT
    xq = _q8(x)                                   # [B, N_IN] fp8
    u_sum = (x - xq.astype(np.float32)).sum(axis=1)   # [BATCH] f32
    w1rm = W1.mean(axis=1)                        # [N_MID] f32
    wa0 = _q8(w1rm)
    wa1 = _q8(AUG_SCALE * (w1rm - wa0.astype(np.float32)))
    va0 = _q8(u_sum)
    va1 = _q8(u_sum / AUG_SCALE)

    # W1 pair tiles [it, p, q, m] (+ aug plane) -> SBUF layout [g,p,it,q,ml]
    w1p = np.zeros((ITk, P, 2, N_MID), dtype=_FP8)
    w1p[:IT - 1] = _dr_pairs(_q8(W1).T)
    if not g1_dve:
        w1p[IT - 1, 0, 0, :] = wa0
        w1p[IT - 1, 0, 1, :] = wa1
    w1T = np.ascontiguousarray(
        w1p.reshape(ITk, P, 2, 16, MG * P).transpose(3, 1, 0, 2, 4))

    # W2 pair tiles [t, p, q, o] -> chunked SBUF layout [c, p, tt, q, o]
    w2p = _dr_pairs(_q8(W2).T)                    # [32, 128, 2, N_OUT]
    w2T = np.ascontiguousarray(
        w2p.reshape(8, 4, P, 2, N_OUT).transpose(0, 2, 1, 3, 4))

    b1s = np.ascontiguousarray(
        np.asarray(b1, dtype=np.float32).reshape(N_MID // P, P).T)
    b2s = np.ascontiguousarray(
        np.asarray(b2, dtype=np.float32).reshape(N_OUT // P, P).T)

    w1rms = np.ascontiguousarray(w1rm.reshape(N_MID // P, P).T)
    in_maps = []
    for c in range(N_CORES):
        sl = slice(c * B, (c + 1) * B)
        xTc = np.zeros((ITk, P, 2, B), dtype=_FP8)
        xTc[:IT - 1] = _dr_pairs(np.ascontiguousarray(xq[sl].T))
        if not g1_dve:
            xTc[IT - 1, 0, 0, :] = va0[sl]
            xTc[IT - 1, 0, 1, :] = va1[sl]
        xTc = np.ascontiguousarray(xTc.transpose(1, 0, 2, 3))  # [p, it, q, b]
        im = {"xT": xTc, "w1T": w1T, "w2T": w2T, "b1s": b1s, "b2s": b2s}
        if g1_dve:
            im["sbc"] = np.ascontiguousarray(np.broadcast_to(
                u_sum[sl].astype(ml_dtypes.bfloat16)[None, :], (P, B)))
            im["w1rms"] = w1rms
        in_maps.append(im)
    return in_maps


def _run(x, W1, b1, W2, b2, trace=False, g1_dt=G1_DT, g2_dt=G2_DT):
    from concourse.bass_utils import run_bass_kernel_spmd
    if g2_dt == "fp8drf" and np.any(np.asarray(b2)):
        g2_dt = "fp8dr"  # flipped path has no b2 port; b2==0 in practice
    nc = _build(g1_dt, g2_dt)
    in_maps = _prep_inputs(x, W1, b1, W2, b2, g1_dt, g2_dt)
    res = run_bass_kernel_spmd(nc, in_maps, core_ids=list(range(N_CORES)),
                               trace=trace)
    if g2_dt == "fp8drf":
        out = np.concatenate(
            [np.asarray(res.results[c]["outF"], dtype=np.float32)
             for c in range(N_CORES)], axis=0)
    else:
        out = np.concatenate(
            [res.results[c]["outT"].T for c in range(N_CORES)], axis=0)
    return np.ascontiguousarray(out, dtype=np.float32), res


def kernel(x, W1, b1, W2, b2):
    out, _ = _run(x, W1, b1, W2, b2, trace=False)
    return out
